# revision 1
# baseline (speedup 1.0000x reference)
"""CFR_flow_t_align (DeMFI) forward-warp kernel for 8x Trainium2 NeuronCores.

Strategy (v2)
-------------
Pure data-parallel over batch N: core i processes image i.  Per image: two
forward warps (splats) + elementwise combine.  The splat is a dense masked
accumulation over integer displacement buckets (A = row shift, B = col
shift); per occupied cell:   dacc_A[:, x+B] += (rowmask_A * v3) * colpsi_B.

v2 over baseline:
  * Per-(core, band) displacement sets; shared (unguarded) instruction stream
    over the union of cores' A-buckets -- per-core masks (from each core's own
    t value) zero out non-applicable work automatically.  Only the inner
    B-chunk multiplies/accumulates are guarded per core (DVE-only tc.If).
  * B-runs processed as chunked broadcast multiplies + single-instruction
    overlapped accumulates (DVE streams in order, so windows 2 columns apart
    accumulate correctly within one op).  Even/odd parity split into two dacc
    slots keeps every pair op 4-byte aligned => DVE 2x fp16 mode.
  * fp16 canvas/planes; one canvas in SBUF, warp-0 canvas spilled to HBM.
  * Row rotation via HWDGE SBUF->SBUF DMA into a scratch tile (+ ACT zeroing
    of the scratch), canvas accumulation on DVE.
"""

import math

import numpy as np

P = 128  # SBUF partitions
BIGC = 1.5 * float(1 << 23)  # keeps x+BIGC in [2^23, 2^24) where f32 ulp = 1
CHUNK = 2  # B-values per batched multiply


# ---------------------------------------------------------------------------
# Host-side plan derivation (sizing/occupancy only -- all math runs on device)
# ---------------------------------------------------------------------------

def _derive_plan(flow_01, flow_10, t_value):
    n, _, H, W = flow_01.shape
    t = np.asarray(t_value, dtype=np.float32).reshape(n)
    NB = (H + P - 1) // P
    a_min = b_min = 10 ** 9
    a_max = b_max = -(10 ** 9)
    warps = []  # [w][band] -> {'union': [(A, {ci: (B0,B1)})...], 'cb': (cb0,cb1)}
    for w in range(2):
        bands_u = []
        for b in range(NB):
            bands_u.append({})
        for i in range(n):
            s = np.float32(t[i]) if w == 0 else np.float32(1.0) - np.float32(t[i])
            flow = np.asarray(flow_01[i] if w == 0 else flow_10[i], np.float32)
            xs = np.float32(s) * flow[1]
            ys = np.float32(s) * flow[0]
            afl = np.floor(xs).astype(np.int64)
            bfl = np.floor(ys).astype(np.int64)
            for b in range(NB):
                sl = slice(P * b, min(P * b + P, H))
                keys = np.unique((afl[sl] + 64) * 512 + (bfl[sl] + 64))
                a_items = {}
                for k in keys:
                    a = int(k // 512) - 64
                    bb = int(k % 512) - 64
                    for da in (0, 1):  # dilate by the 2x2 tap footprint
                        lo, hi = a_items.get(a + da, (10 ** 9, -(10 ** 9)))
                        a_items[a + da] = (min(lo, bb), max(hi, bb + 1))
                for a, (lo, hi) in a_items.items():
                    bands_u[b].setdefault(a, {})[i] = (lo, hi)
        out_bands = []
        for b in range(NB):
            union = []
            cb0, cb1 = 10 ** 9, -(10 ** 9)
            for a in sorted(bands_u[b]):
                cores = bands_u[b][a]
                union.append((a, cores))
                lo = min(v[0] for v in cores.values())
                hi = max(v[1] for v in cores.values())
                cb0 = min(cb0, lo)
                cb1 = max(cb1, hi)
                a_min = min(a_min, a)
                a_max = max(a_max, a)
            b_min = min(b_min, cb0)
            b_max = max(b_max, cb1)
            out_bands.append({"union": union, "cb": (cb0, cb1)})
        warps.append(out_bands)
    return warps, (a_min, a_max, b_min, b_max)


# ---------------------------------------------------------------------------
# Device program
# ---------------------------------------------------------------------------

def _build_program(H, W, warps, ranges, n_cores, repeat=1):
    import bass_rust
    import concourse.bacc as bacc
    import concourse.mybir as mybir
    import concourse.tile as tile

    f32 = mybir.dt.float32
    fp16 = mybir.dt.float16
    Alu = mybir.AluOpType
    Act = mybir.ActivationFunctionType

    a_min, a_max, b_min, b_max = ranges
    MARG_L = 2 * math.ceil(max(0, -b_min) / 2)
    W2D = MARG_L + W + max(0, b_max) + 2
    W2D = (W2D + 7) // 8 * 8          # dacc slot width (even)
    W2C = W2D + 2                      # canvas width (odd slot shifted +1)
    ROW_OFF = 32 * math.ceil(max(0, -a_min) / 32)
    HC = (ROW_OFF + H + max(0, a_max) + 1 + P - 1) // P * P
    HCB = HC // P
    NB = (H + P - 1) // P
    band_rows = [min(P, H - P * b) for b in range(NB)]
    assert all(r % 32 == 0 for r in band_rows), band_rows

    n_cache = 1
    for w in range(2):
        for b in range(NB):
            cb0, cb1 = warps[w][b]["cb"]
            n_cache = max(n_cache, cb1 - cb0 + 1)

    nc = bacc.Bacc("TRN2", enable_partition_id=True)
    d_f01 = nc.dram_tensor("flow01", [2, H, W], f32, kind="ExternalInput")
    d_f10 = nc.dram_tensor("flow10", [2, H, W], f32, kind="ExternalInput")
    d_tv = nc.dram_tensor("tv", [P, 1], f32, kind="ExternalInput")
    d_out0 = nc.dram_tensor("out0", [2, H, W], f32, kind="ExternalOutput")
    d_out1 = nc.dram_tensor("out1", [2, H, W], f32, kind="ExternalOutput")

    def strided(ap, offset, dims):
        part = ap.ap[0]
        return bass_rust.AP(ap.tensor, offset,
                            [list(part)] + [list(d) for d in dims])

    with tile.TileContext(nc) as tc:
        with (
            tc.tile_pool(name="dram", bufs=1, space="DRAM") as dram_pool,
            tc.tile_pool(name="const", bufs=1) as const_pool,
            tc.tile_pool(name="canvas", bufs=1) as canvas_pool,
            tc.tile_pool(name="v3", bufs=1) as v3_pool,
            tc.tile_pool(name="planes", bufs=1) as planes_pool,
            tc.tile_pool(name="trans", bufs=1) as trans_pool,
            tc.tile_pool(name="cache", bufs=1) as cache_pool,
            tc.tile_pool(name="work", bufs=1) as work_pool,
            tc.tile_pool(name="cv0", bufs=1) as cv0_pool,
        ):
            c0_hbm = dram_pool.tile([HC, 3, W2C], fp16)

            # ---- scalars (tv arrives replicated across partitions) -------
            t_sb = const_pool.tile([P, 1], f32)
            nc.sync.dma_start(out=t_sb[:, :], in_=d_tv[:, :])
            omt = const_pool.tile([P, 1], f32)   # 1 - t
            nc.vector.tensor_scalar(out=omt[:, :], in0=t_sb[:, :], scalar1=-1.0,
                                    scalar2=1.0, op0=Alu.mult, op1=Alu.add)
            al0 = const_pool.tile([P, 1], f32)   # -(1-t)*t
            nc.vector.tensor_tensor(out=al0[:, :], in0=omt[:, :], in1=t_sb[:, :], op=Alu.mult)
            nc.vector.tensor_scalar(out=al0[:, :], in0=al0[:, :], scalar1=-1.0, scalar2=None, op0=Alu.mult)
            al1 = const_pool.tile([P, 1], f32)   # t^2
            nc.vector.tensor_tensor(out=al1[:, :], in0=t_sb[:, :], in1=t_sb[:, :], op=Alu.mult)
            be0 = const_pool.tile([P, 1], f32)   # (1-t)^2
            nc.vector.tensor_tensor(out=be0[:, :], in0=omt[:, :], in1=omt[:, :], op=Alu.mult)
            neg1 = const_pool.tile([P, 1], f32)
            nc.vector.memset(neg1[:, :], -1.0)

            pid = nc.vector.partition_id()

            # fixed work tiles (no allocation inside If regions)
            canvas = canvas_pool.tile([P, HCB, 3, W2C], fp16)
            v3f = v3_pool.tile([P, 2, W], f32)
            v3h = v3_pool.tile([P, 3, W], fp16)
            xs = trans_pool.tile([P, W], f32, tag="xs")
            ys = trans_pool.tile([P, W], f32, tag="ys")
            tfa = trans_pool.tile([P, W], f32, tag="tfa")
            tfb = trans_pool.tile([P, W], f32, tag="tfb")
            afl = planes_pool.tile([P, W], fp16, tag="afl")
            bfl = planes_pool.tile([P, W], fp16, tag="bfl")
            wr1 = planes_pool.tile([P, W], fp16, tag="wr1")
            wr2 = planes_pool.tile([P, W], fp16, tag="wr2")
            wc1 = planes_pool.tile([P, W], fp16, tag="wc1")
            wc2 = planes_pool.tile([P, W], fp16, tag="wc2")
            psic = cache_pool.tile([P, n_cache, W], fp16)
            m1 = planes_pool.tile([P, W], fp16, tag="m1")
            mc = planes_pool.tile([P, W], fp16, tag="mc")
            rowg = work_pool.tile([P, 3, W], fp16, tag="rowg")
            tmp = work_pool.tile([P, CHUNK, 3, W], fp16, tag="tmp")
            dacc0 = work_pool.tile([P, 2, 3, W2D], fp16, tag="dacc0")
            dacc1 = work_pool.tile([P, 2, 3, W2D], fp16, tag="dacc1")
            daccs = [dacc0, dacc1]
            scr = work_pool.tile([P, 2, 3, W2D], fp16, tag="scr")

            def floor_frac_weights(src, fl_t, w1_t, w2_t):
                # r = round_to_nearest_even(src); floor = r - (r > src)
                nc.vector.tensor_scalar(out=tfa[:, :], in0=src[:, :], scalar1=BIGC,
                                        scalar2=BIGC, op0=Alu.add, op1=Alu.subtract)
                nc.vector.tensor_tensor(out=tfb[:, :], in0=tfa[:, :], in1=src[:, :], op=Alu.is_gt)
                nc.vector.tensor_tensor(out=tfa[:, :], in0=tfa[:, :], in1=tfb[:, :], op=Alu.subtract)
                nc.vector.tensor_copy(fl_t[:, :], tfa[:, :])
                # fx = src - floor ;  w1 = exp(-fx^2) ; w2 = exp(-(fx-1)^2)
                nc.vector.tensor_tensor(out=tfa[:, :], in0=src[:, :], in1=tfa[:, :], op=Alu.subtract)
                nc.scalar.activation(tfb[:, :], tfa[:, :], Act.Square)
                nc.scalar.activation(w1_t[:, :], tfb[:, :], Act.Exp, scale=-1.0)
                nc.scalar.activation(tfb[:, :], tfa[:, :], Act.Square, bias=neg1[:, 0:1])
                nc.scalar.activation(w2_t[:, :], tfb[:, :], Act.Exp, scale=-1.0)

            def do_warp(w, flow_dram, s_ap):
                nc.vector.memset(canvas[:, :, :, :], 0.0)
                for b in range(NB):
                    rows = band_rows[b]
                    plan = warps[w][b]
                    union = plan["union"]
                    cb0, cb1 = plan["cb"]

                    nc.sync.dma_start(out=v3f[0:rows, 0, :], in_=flow_dram[0, P * b:P * b + rows, :])
                    nc.sync.dma_start(out=v3f[0:rows, 1, :], in_=flow_dram[1, P * b:P * b + rows, :])
                    p_ = rows
                    while p_ < P:  # legal engine partition windows: 0/32/64/96
                        ln = {0: P, 32: 32, 64: 64, 96: 32}[p_]
                        nc.vector.memset(v3f[p_:p_ + ln, :, :], 0.0)
                        nc.vector.memset(v3h[p_:p_ + ln, :, :], 0.0)
                        p_ += ln
                    nc.vector.tensor_copy(v3h[0:rows, 0:2, :], v3f[0:rows, :, :])
                    nc.vector.memset(v3h[0:rows, 2, :], 1.0)

                    nc.vector.tensor_scalar(out=xs[:, :], in0=v3f[:, 1, :], scalar1=s_ap, scalar2=None, op0=Alu.mult)
                    nc.vector.tensor_scalar(out=ys[:, :], in0=v3f[:, 0, :], scalar1=s_ap, scalar2=None, op0=Alu.mult)
                    floor_frac_weights(xs, afl, wr1, wr2)
                    floor_frac_weights(ys, bfl, wc1, wc2)

                    # column psi cache for the union B-range (shared stream;
                    # per-core correctness comes from each core's own bfl/wc)
                    for j, B in enumerate(range(cb0, cb1 + 1)):
                        pj = psic[:, j, :]
                        nc.vector.scalar_tensor_tensor(
                            out=m1[:, :], in0=bfl[:, :], scalar=float(B),
                            in1=wc1[:, :], op0=Alu.is_equal, op1=Alu.mult)
                        nc.vector.scalar_tensor_tensor(
                            out=pj, in0=bfl[:, :], scalar=float(B - 1),
                            in1=wc2[:, :], op0=Alu.is_equal, op1=Alu.mult)
                        nc.vector.tensor_tensor(out=pj, in0=pj, in1=m1[:, :], op=Alu.add)

                    for ia, (A, cores) in enumerate(union):
                        nc.vector.scalar_tensor_tensor(
                            out=m1[:, :], in0=afl[:, :], scalar=float(A),
                            in1=wr1[:, :], op0=Alu.is_equal, op1=Alu.mult)
                        nc.vector.scalar_tensor_tensor(
                            out=mc[:, :], in0=afl[:, :], scalar=float(A - 1),
                            in1=wr2[:, :], op0=Alu.is_equal, op1=Alu.mult)
                        nc.vector.tensor_tensor(out=mc[:, :], in0=mc[:, :], in1=m1[:, :], op=Alu.add)
                        nc.vector.tensor_tensor(
                            out=rowg[:, :, :],
                            in0=mc[:, :].rearrange("p (o w) -> p o w", o=1).to_broadcast([P, 3, W]),
                            in1=v3h[:, :, :], op=Alu.mult)

                        dacc = daccs[ia % 2]
                        nc.vector.memset(dacc[:, :, :, :], 0.0)
                        for ci, (B0, B1) in sorted(cores.items()):
                            with tc.If(pid == ci):
                                for eo in (0, 1):
                                    Be0 = B0 + ((B0 % 2) != eo)
                                    ne = max(0, (B1 - Be0) // 2 + 1)
                                    c0 = 0
                                    while c0 < ne:
                                        nchunk = min(CHUNK, ne - c0)
                                        jj = (Be0 + 2 * c0) - cb0
                                        in0 = strided(rowg[:, :, :], 0,
                                                      [(0, nchunk), (W, 3), (1, W)])
                                        in1 = strided(psic[:, 0, :], jj * W,
                                                      [(2 * W, nchunk), (0, 3), (1, W)])
                                        nc.vector.tensor_tensor(
                                            out=tmp[:, 0:nchunk, :, :], in0=in0,
                                            in1=in1, op=Alu.mult)
                                        off = eo * (3 * W2D) + (MARG_L + Be0 + 2 * c0 - eo)
                                        dst = strided(dacc[:, 0, 0, :], off,
                                                      [(2, nchunk), (W2D, 3), (1, W)])
                                        nc.vector.tensor_tensor(
                                            out=dst, in0=dst,
                                            in1=tmp[:, 0:nchunk, :, :], op=Alu.add)
                                        c0 += nchunk

                        # row-shift via HWDGE SBUF->SBUF DMA + DVE accumulate
                        s0 = P * b + A + ROW_OFF
                        jlo, p0 = divmod(s0, P)
                        len1 = min(rows, P - p0)
                        pieces = [(p0, jlo, 0, len1)]
                        if len1 < rows:
                            pieces.append((0, jlo + 1, len1, rows - len1))
                        for pdst, jb, srow, ln in pieces:
                            if not (pdst == 0 and ln == P):
                                nc.scalar.memzero(scr[:, :, :, :])
                            nc.sync.dma_start(out=scr[pdst:pdst + ln, :, :, :],
                                              in_=dacc[srow:srow + ln, :, :, :])
                            for eo in (0, 1):
                                dstc = canvas[:, jb, :, eo:eo + W2D]
                                nc.vector.tensor_tensor(
                                    out=dstc, in0=dstc, in1=scr[:, eo, :, :],
                                    op=Alu.add)

                if w == 0:  # spill canvas0 to HBM (overlaps with warp 1 start)
                    for jb in range(HCB):
                        nc.sync.dma_start(out=c0_hbm[P * jb:P * jb + P, :, :],
                                          in_=canvas[:, jb, :, :])

            for _rep in range(repeat):
                do_warp(0, d_f01, t_sb[:, 0:1])
                do_warp(1, d_f10, omt[:, 0:1])

                # ---- combine ---------------------------------------------
                sl = slice(MARG_L, MARG_L + W)
                for jb in range(HCB):
                    lo = P * jb
                    o_lo = max(0, lo - ROW_OFF)
                    o_hi = min(H, lo + P - ROW_OFF)
                    if o_lo >= o_hi:
                        continue
                    cv0 = cv0_pool.tile([P, 3, W2C], fp16, tag="cv0")
                    nc.sync.dma_start(out=cv0[:, :, :], in_=c0_hbm[lo:lo + P, :, :])

                    # nhat = (1-t)*n0 + t*n1 + 1 ; m = nhat>1 ; den = nhat - m
                    tn1 = trans_pool.tile([P, W], f32, tag="tfa")
                    nc.vector.tensor_scalar(out=tn1[:, :], in0=canvas[:, jb, 2, sl],
                                            scalar1=t_sb[:, 0:1], scalar2=1.0,
                                            op0=Alu.mult, op1=Alu.add)
                    nhat = trans_pool.tile([P, W], f32, tag="tfb")
                    nc.vector.scalar_tensor_tensor(
                        out=nhat[:, :], in0=cv0[:, 2, sl], scalar=omt[:, 0:1],
                        in1=tn1[:, :], op0=Alu.mult, op1=Alu.add)
                    mgt = trans_pool.tile([P, W], f32, tag="xs")
                    nc.vector.tensor_scalar(out=mgt[:, :], in0=nhat[:, :], scalar1=1.0, scalar2=None, op0=Alu.is_gt)
                    den = trans_pool.tile([P, W], f32, tag="tfa")
                    nc.vector.tensor_tensor(out=den[:, :], in0=nhat[:, :], in1=mgt[:, :], op=Alu.subtract)
                    rec = trans_pool.tile([P, W], f32, tag="ys")
                    nc.vector.reciprocal(rec[:, :], den[:, :])

                    p_lo = o_lo + ROW_OFF - lo
                    p_hi = o_hi + ROW_OFF - lo
                    for c in range(2):
                        u = trans_pool.tile([P, W], f32, tag="tfa")
                        o0 = trans_pool.tile([P, W], f32, tag="o0")
                        o1 = trans_pool.tile([P, W], f32, tag="tfb")
                        nc.vector.tensor_scalar(out=u[:, :], in0=cv0[:, c, sl],
                                                scalar1=al0[:, 0:1], scalar2=None, op0=Alu.mult)
                        nc.vector.scalar_tensor_tensor(
                            out=o0[:, :], in0=canvas[:, jb, c, sl], scalar=al1[:, 0:1],
                            in1=u[:, :], op0=Alu.mult, op1=Alu.add)
                        nc.vector.tensor_tensor(out=o0[:, :], in0=o0[:, :], in1=rec[:, :], op=Alu.mult)
                        nc.vector.tensor_scalar(out=u[:, :], in0=cv0[:, c, sl],
                                                scalar1=be0[:, 0:1], scalar2=None, op0=Alu.mult)
                        nc.vector.scalar_tensor_tensor(
                            out=o1[:, :], in0=canvas[:, jb, c, sl], scalar=al0[:, 0:1],
                            in1=u[:, :], op0=Alu.mult, op1=Alu.add)
                        nc.vector.tensor_tensor(out=o1[:, :], in0=o1[:, :], in1=rec[:, :], op=Alu.mult)
                        nc.sync.dma_start(out=d_out0[c, o_lo:o_hi, :], in_=o0[p_lo:p_hi, :])
                        nc.sync.dma_start(out=d_out1[c, o_lo:o_hi, :], in_=o1[p_lo:p_hi, :])

    nc.finalize()
    return nc


# ---------------------------------------------------------------------------
# Entry point
# ---------------------------------------------------------------------------

def _prepare(flow_01, flow_10, t_value):
    flow_01 = np.ascontiguousarray(np.asarray(flow_01, dtype=np.float32))
    flow_10 = np.ascontiguousarray(np.asarray(flow_10, dtype=np.float32))
    t_value = np.ascontiguousarray(np.asarray(t_value, dtype=np.float32))
    n, _, H, W = flow_01.shape

    warps, ranges = _derive_plan(flow_01, flow_10, t_value)
    nc = _build_program(H, W, warps, ranges, n)

    in_maps = []
    for i in range(n):
        in_maps.append({
            "flow01": flow_01[i],
            "flow10": flow_10[i],
            "tv": np.full((P, 1), t_value[i].reshape(()), dtype=np.float32),
        })
    return nc, in_maps, n


def kernel(flow_01, flow_10, t_value):
    from concourse.bass_utils import run_bass_kernel_spmd

    nc, in_maps, n = _prepare(flow_01, flow_10, t_value)
    res = run_bass_kernel_spmd(nc, in_maps, list(range(n)))
    out0 = np.stack([res.results[i]["out0"] for i in range(n)])
    out1 = np.stack([res.results[i]["out1"] for i in range(n)])
    return out0, out1


def _make_runner(nc, in_maps, n_cores):
    """Mirror bass2jax.run_bass_via_pjrt's multi-core path, but return a
    cached jitted callable (no donation) so repeated timed runs are possible."""
    import jax
    from jax.sharding import Mesh, PartitionSpec, NamedSharding
    from jax.experimental.shard_map import shard_map
    from concourse import bass2jax, mybir

    bass2jax.install_neuronx_cc_hook()
    partition_name = nc.partition_id_tensor.name if nc.partition_id_tensor else None
    in_names, out_names, out_avals, zero_outs = [], [], [], []
    for alloc in nc.m.functions[0].allocations:
        if not isinstance(alloc, mybir.MemoryLocationSet):
            continue
        name = alloc.memorylocations[0].name
        if alloc.kind == "ExternalInput":
            if name != partition_name:
                in_names.append(name)
        elif alloc.kind == "ExternalOutput":
            shape = tuple(alloc.tensor_shape)
            dtype = mybir.dt.np(alloc.dtype)
            out_names.append(name)
            out_avals.append(jax.core.ShapedArray(shape, dtype))
            zero_outs.append(np.zeros(shape, dtype))
    n_params = len(in_names)
    all_in_names = in_names + out_names
    if partition_name is not None:
        all_in_names.append(partition_name)

    def _body(*args):
        operands = list(args)
        if partition_name is not None:
            operands.append(bass2jax.partition_id_tensor())
        return tuple(bass2jax._bass_exec_p.bind(
            *operands,
            out_avals=tuple(out_avals),
            in_names=tuple(all_in_names),
            out_names=tuple(out_names),
            lowering_input_output_aliases=(),
            sim_require_finite=True,
            sim_require_nnan=True,
            nc=nc,
        ))

    devices = jax.devices()[:n_cores]
    mesh = Mesh(np.asarray(devices), ("core",))
    in_specs = (PartitionSpec("core"),) * (n_params + len(out_names))
    out_specs = (PartitionSpec("core"),) * len(out_names)
    fn = jax.jit(shard_map(_body, mesh=mesh, in_specs=in_specs,
                           out_specs=out_specs, check_rep=False))
    per_core = [[np.asarray(m[nm]) for nm in in_names] for m in in_maps]
    concat_in = [np.concatenate([per_core[c][i] for c in range(n_cores)], axis=0)
                 for i in range(n_params)]
    concat_zero = [np.concatenate([z] * n_cores, axis=0) for z in zero_outs]
    sh = NamedSharding(mesh, PartitionSpec("core"))
    concat_in = [jax.device_put(a, sh) for a in concat_in]
    concat_zero = [jax.device_put(a, sh) for a in concat_zero]
    return fn, concat_in, concat_zero


def bench(flow_01, flow_10, t_value, iters=8):
    """Wall-clock the jitted SPMD executable; returns min per-iter ns."""
    import time
    import jax

    nc, in_maps, n = _prepare(flow_01, flow_10, t_value)
    fn, concat_in, concat_zero = _make_runner(nc, in_maps, n)
    out = fn(*concat_in, *concat_zero)
    jax.block_until_ready(out)
    times = []
    for _ in range(iters):
        t0 = time.perf_counter()
        out = fn(*concat_in, *concat_zero)
        jax.block_until_ready(out)
        times.append(time.perf_counter() - t0)
    print("bench iters (ms):", [round(t * 1e3, 2) for t in times])
    return int(min(times) * 1e9)



# revision 14
# speedup vs baseline: 1.1101x; 1.1101x over previous
"""CFR_flow_t_align (DeMFI) forward-warp kernel for 8x Trainium2 NeuronCores.

Strategy (v2)
-------------
Pure data-parallel over batch N: core i processes image i.  Per image: two
forward warps (splats) + elementwise combine.  The splat is a dense masked
accumulation over integer displacement buckets (A = row shift, B = col
shift); per occupied cell:   dacc_A[:, x+B] += (rowmask_A * v3) * colpsi_B.

v2 over baseline:
  * Per-(core, band) displacement sets; shared (unguarded) instruction stream
    over the union of cores' A-buckets -- per-core masks (from each core's own
    t value) zero out non-applicable work automatically.  Only the inner
    B-chunk multiplies/accumulates are guarded per core (DVE-only tc.If).
  * B-runs processed as chunked broadcast multiplies + single-instruction
    overlapped accumulates (DVE streams in order, so windows 2 columns apart
    accumulate correctly within one op).  Even/odd parity split into two dacc
    slots keeps every pair op 4-byte aligned => DVE 2x fp16 mode.
  * fp16 canvas/planes; one canvas in SBUF, warp-0 canvas spilled to HBM.
  * Row rotation via HWDGE SBUF->SBUF DMA into a scratch tile (+ ACT zeroing
    of the scratch), canvas accumulation on DVE.
  * v2.5: dacc zeroing moved to the ACT engine (scalar.memzero, same pattern
    as the scr zeroing) -- removes ~300 large memsets from the DVE critical
    path.
"""

import math

import numpy as np

P = 128  # SBUF partitions
BIGC = 1.5 * float(1 << 23)  # keeps x+BIGC in [2^23, 2^24) where f32 ulp = 1
CHUNK = 2  # B-values per batched multiply


# ---------------------------------------------------------------------------
# Host-side plan derivation (sizing/occupancy only -- all math runs on device)
# ---------------------------------------------------------------------------

def _derive_plan(flow_01, flow_10, t_value):
    n, _, H, W = flow_01.shape
    t = np.asarray(t_value, dtype=np.float32).reshape(n)
    NB = (H + P - 1) // P
    a_min = b_min = 10 ** 9
    a_max = b_max = -(10 ** 9)
    warps = []  # [w][band] -> {'union': [(A, {ci: (B0,B1)})...], 'cb': (cb0,cb1)}
    for w in range(2):
        bands_u = []
        for b in range(NB):
            bands_u.append({})
        for i in range(n):
            s = np.float32(t[i]) if w == 0 else np.float32(1.0) - np.float32(t[i])
            flow = np.asarray(flow_01[i] if w == 0 else flow_10[i], np.float32)
            xs = np.float32(s) * flow[1]
            ys = np.float32(s) * flow[0]
            afl = np.floor(xs).astype(np.int64)
            bfl = np.floor(ys).astype(np.int64)
            for b in range(NB):
                sl = slice(P * b, min(P * b + P, H))
                keys = np.unique((afl[sl] + 64) * 512 + (bfl[sl] + 64))
                a_items = {}
                for k in keys:
                    a = int(k // 512) - 64
                    bb = int(k % 512) - 64
                    for da in (0, 1):  # dilate by the 2x2 tap footprint
                        lo, hi = a_items.get(a + da, (10 ** 9, -(10 ** 9)))
                        a_items[a + da] = (min(lo, bb), max(hi, bb + 1))
                for a, (lo, hi) in a_items.items():
                    bands_u[b].setdefault(a, {})[i] = (lo, hi)
        out_bands = []
        for b in range(NB):
            union = []
            cb0, cb1 = 10 ** 9, -(10 ** 9)
            for a in sorted(bands_u[b]):
                cores = bands_u[b][a]
                union.append((a, cores))
                lo = min(v[0] for v in cores.values())
                hi = max(v[1] for v in cores.values())
                cb0 = min(cb0, lo)
                cb1 = max(cb1, hi)
                a_min = min(a_min, a)
                a_max = max(a_max, a)
            b_min = min(b_min, cb0)
            b_max = max(b_max, cb1)
            out_bands.append({"union": union, "cb": (cb0, cb1)})
        warps.append(out_bands)
    return warps, (a_min, a_max, b_min, b_max)


# ---------------------------------------------------------------------------
# Device program
# ---------------------------------------------------------------------------

def _build_program(H, W, warps, ranges, n_cores, repeat=1):
    import bass_rust
    import concourse.bacc as bacc
    import concourse.mybir as mybir
    import concourse.tile as tile

    f32 = mybir.dt.float32
    fp16 = mybir.dt.float16
    Alu = mybir.AluOpType
    Act = mybir.ActivationFunctionType

    a_min, a_max, b_min, b_max = ranges
    MARG_L = 2 * math.ceil(max(0, -b_min) / 2)
    W2D = MARG_L + W + max(0, b_max) + 2
    W2D = (W2D + 7) // 8 * 8          # dacc slot width (even)
    W2C = W2D + 2                      # canvas width (odd slot shifted +1)
    ROW_OFF = 32 * math.ceil(max(0, -a_min) / 32)
    HC = (ROW_OFF + H + max(0, a_max) + 1 + P - 1) // P * P
    HCB = HC // P
    NB = (H + P - 1) // P
    band_rows = [min(P, H - P * b) for b in range(NB)]
    assert all(r % 32 == 0 for r in band_rows), band_rows

    n_cache = 1
    for w in range(2):
        for b in range(NB):
            cb0, cb1 = warps[w][b]["cb"]
            n_cache = max(n_cache, cb1 - cb0 + 1)

    nc = bacc.Bacc("TRN2", enable_partition_id=True)
    d_f01 = nc.dram_tensor("flow01", [2, H, W], f32, kind="ExternalInput")
    d_f10 = nc.dram_tensor("flow10", [2, H, W], f32, kind="ExternalInput")
    d_tv = nc.dram_tensor("tv", [P, 1], f32, kind="ExternalInput")
    d_out0 = nc.dram_tensor("out0", [2, H, W], f32, kind="ExternalOutput")
    d_out1 = nc.dram_tensor("out1", [2, H, W], f32, kind="ExternalOutput")

    def strided(ap, offset, dims):
        part = ap.ap[0]
        return bass_rust.AP(ap.tensor, offset,
                            [list(part)] + [list(d) for d in dims])

    with tile.TileContext(nc) as tc:
        with (
            tc.tile_pool(name="dram", bufs=1, space="DRAM") as dram_pool,
            tc.tile_pool(name="const", bufs=1) as const_pool,
            tc.tile_pool(name="canvas", bufs=1) as canvas_pool,
            tc.tile_pool(name="v3", bufs=1) as v3_pool,
            tc.tile_pool(name="planes", bufs=1) as planes_pool,
            tc.tile_pool(name="trans", bufs=1) as trans_pool,
            tc.tile_pool(name="cache", bufs=1) as cache_pool,
            tc.tile_pool(name="work", bufs=1) as work_pool,
            tc.tile_pool(name="cv0", bufs=1) as cv0_pool,
        ):
            c0_hbm = dram_pool.tile([HC, 3, W2C], fp16)

            # ---- scalars (tv arrives replicated across partitions) -------
            t_sb = const_pool.tile([P, 1], f32)
            nc.sync.dma_start(out=t_sb[:, :], in_=d_tv[:, :])
            omt = const_pool.tile([P, 1], f32)   # 1 - t
            nc.vector.tensor_scalar(out=omt[:, :], in0=t_sb[:, :], scalar1=-1.0,
                                    scalar2=1.0, op0=Alu.mult, op1=Alu.add)
            al0 = const_pool.tile([P, 1], f32)   # -(1-t)*t
            nc.vector.tensor_tensor(out=al0[:, :], in0=omt[:, :], in1=t_sb[:, :], op=Alu.mult)
            nc.vector.tensor_scalar(out=al0[:, :], in0=al0[:, :], scalar1=-1.0, scalar2=None, op0=Alu.mult)
            al1 = const_pool.tile([P, 1], f32)   # t^2
            nc.vector.tensor_tensor(out=al1[:, :], in0=t_sb[:, :], in1=t_sb[:, :], op=Alu.mult)
            be0 = const_pool.tile([P, 1], f32)   # (1-t)^2
            nc.vector.tensor_tensor(out=be0[:, :], in0=omt[:, :], in1=omt[:, :], op=Alu.mult)
            neg1 = const_pool.tile([P, 1], f32)
            nc.vector.memset(neg1[:, :], -1.0)

            pid = nc.vector.partition_id()

            # fixed work tiles (no allocation inside If regions)
            canvas = canvas_pool.tile([P, HCB, 3, W2C], fp16)
            v3f = v3_pool.tile([P, 2, W], f32)
            v3h = v3_pool.tile([P, 3, W], fp16)
            xs = trans_pool.tile([P, W], f32, tag="xs")
            ys = trans_pool.tile([P, W], f32, tag="ys")
            tfa = trans_pool.tile([P, W], f32, tag="tfa")
            tfb = trans_pool.tile([P, W], f32, tag="tfb")
            afl = planes_pool.tile([P, W], fp16, tag="afl")
            bfl = planes_pool.tile([P, W], fp16, tag="bfl")
            wr1 = planes_pool.tile([P, W], fp16, tag="wr1")
            wr2 = planes_pool.tile([P, W], fp16, tag="wr2")
            wc1 = planes_pool.tile([P, W], fp16, tag="wc1")
            wc2 = planes_pool.tile([P, W], fp16, tag="wc2")
            psic = cache_pool.tile([P, n_cache, W], fp16)
            m1 = planes_pool.tile([P, W], fp16, tag="m1")
            mc = planes_pool.tile([P, W], fp16, tag="mc")
            rowg = work_pool.tile([P, 3, W], fp16, tag="rowg")
            tmp = work_pool.tile([P, CHUNK, 3, W], fp16, tag="tmp")
            dacc0 = work_pool.tile([P, 2, 3, W2D], fp16, tag="dacc0")
            dacc1 = work_pool.tile([P, 2, 3, W2D], fp16, tag="dacc1")
            daccs = [dacc0, dacc1]
            scr = work_pool.tile([P, 2, 3, W2D], fp16, tag="scr")

            def floor_frac_weights(src, fl_t, w1_t, w2_t):
                # r = round_to_nearest_even(src); floor = r - (r > src)
                nc.vector.tensor_scalar(out=tfa[:, :], in0=src[:, :], scalar1=BIGC,
                                        scalar2=BIGC, op0=Alu.add, op1=Alu.subtract)
                nc.vector.tensor_tensor(out=tfb[:, :], in0=tfa[:, :], in1=src[:, :], op=Alu.is_gt)
                nc.vector.tensor_tensor(out=tfa[:, :], in0=tfa[:, :], in1=tfb[:, :], op=Alu.subtract)
                nc.vector.tensor_copy(fl_t[:, :], tfa[:, :])
                # fx = src - floor ;  w1 = exp(-fx^2) ; w2 = exp(-(fx-1)^2)
                nc.vector.tensor_tensor(out=tfa[:, :], in0=src[:, :], in1=tfa[:, :], op=Alu.subtract)
                nc.scalar.activation(tfb[:, :], tfa[:, :], Act.Square)
                nc.scalar.activation(w1_t[:, :], tfb[:, :], Act.Exp, scale=-1.0)
                nc.scalar.activation(tfb[:, :], tfa[:, :], Act.Square, bias=neg1[:, 0:1])
                nc.scalar.activation(w2_t[:, :], tfb[:, :], Act.Exp, scale=-1.0)

            def do_warp(w, flow_dram, s_ap):
                nc.vector.memset(canvas[:, :, :, :], 0.0)
                for b in range(NB):
                    rows = band_rows[b]
                    plan = warps[w][b]
                    union = plan["union"]
                    cb0, cb1 = plan["cb"]

                    nc.sync.dma_start(out=v3f[0:rows, 0, :], in_=flow_dram[0, P * b:P * b + rows, :])
                    nc.sync.dma_start(out=v3f[0:rows, 1, :], in_=flow_dram[1, P * b:P * b + rows, :])
                    p_ = rows
                    while p_ < P:  # legal engine partition windows: 0/32/64/96
                        ln = {0: P, 32: 32, 64: 64, 96: 32}[p_]
                        nc.vector.memset(v3f[p_:p_ + ln, :, :], 0.0)
                        nc.vector.memset(v3h[p_:p_ + ln, :, :], 0.0)
                        p_ += ln
                    nc.vector.tensor_copy(v3h[0:rows, 0:2, :], v3f[0:rows, :, :])
                    nc.vector.memset(v3h[0:rows, 2, :], 1.0)

                    nc.vector.tensor_scalar(out=xs[:, :], in0=v3f[:, 1, :], scalar1=s_ap, scalar2=None, op0=Alu.mult)
                    nc.vector.tensor_scalar(out=ys[:, :], in0=v3f[:, 0, :], scalar1=s_ap, scalar2=None, op0=Alu.mult)
                    floor_frac_weights(xs, afl, wr1, wr2)
                    floor_frac_weights(ys, bfl, wc1, wc2)

                    # column psi cache for the union B-range (shared stream;
                    # per-core correctness comes from each core's own bfl/wc)
                    for j, B in enumerate(range(cb0, cb1 + 1)):
                        pj = psic[:, j, :]
                        nc.vector.scalar_tensor_tensor(
                            out=m1[:, :], in0=bfl[:, :], scalar=float(B),
                            in1=wc1[:, :], op0=Alu.is_equal, op1=Alu.mult)
                        nc.vector.scalar_tensor_tensor(
                            out=pj, in0=bfl[:, :], scalar=float(B - 1),
                            in1=wc2[:, :], op0=Alu.is_equal, op1=Alu.mult)
                        nc.vector.tensor_tensor(out=pj, in0=pj, in1=m1[:, :], op=Alu.add)

                    for ia, (A, cores) in enumerate(union):
                        nc.vector.scalar_tensor_tensor(
                            out=m1[:, :], in0=afl[:, :], scalar=float(A),
                            in1=wr1[:, :], op0=Alu.is_equal, op1=Alu.mult)
                        nc.vector.scalar_tensor_tensor(
                            out=mc[:, :], in0=afl[:, :], scalar=float(A - 1),
                            in1=wr2[:, :], op0=Alu.is_equal, op1=Alu.mult)
                        nc.vector.tensor_tensor(out=mc[:, :], in0=mc[:, :], in1=m1[:, :], op=Alu.add)
                        nc.vector.tensor_tensor(
                            out=rowg[:, :, :],
                            in0=mc[:, :].rearrange("p (o w) -> p o w", o=1).to_broadcast([P, 3, W]),
                            in1=v3h[:, :, :], op=Alu.mult)

                        dacc = daccs[ia % 2]
                        nc.scalar.memzero(dacc[:, :, :, :])
                        for ci, (B0, B1) in sorted(cores.items()):
                            with tc.If(pid == ci):
                                for eo in (0, 1):
                                    Be0 = B0 + ((B0 % 2) != eo)
                                    ne = max(0, (B1 - Be0) // 2 + 1)
                                    c0 = 0
                                    while c0 < ne:
                                        nchunk = min(CHUNK, ne - c0)
                                        jj = (Be0 + 2 * c0) - cb0
                                        in0 = strided(rowg[:, :, :], 0,
                                                      [(0, nchunk), (W, 3), (1, W)])
                                        in1 = strided(psic[:, 0, :], jj * W,
                                                      [(2 * W, nchunk), (0, 3), (1, W)])
                                        nc.vector.tensor_tensor(
                                            out=tmp[:, 0:nchunk, :, :], in0=in0,
                                            in1=in1, op=Alu.mult)
                                        off = eo * (3 * W2D) + (MARG_L + Be0 + 2 * c0 - eo)
                                        dst = strided(dacc[:, 0, 0, :], off,
                                                      [(2, nchunk), (W2D, 3), (1, W)])
                                        nc.vector.tensor_tensor(
                                            out=dst, in0=dst,
                                            in1=tmp[:, 0:nchunk, :, :], op=Alu.add)
                                        c0 += nchunk

                        # row-shift via HWDGE SBUF->SBUF DMA + DVE accumulate
                        s0 = P * b + A + ROW_OFF
                        jlo, p0 = divmod(s0, P)
                        len1 = min(rows, P - p0)
                        pieces = [(p0, jlo, 0, len1)]
                        if len1 < rows:
                            pieces.append((0, jlo + 1, len1, rows - len1))
                        for pdst, jb, srow, ln in pieces:
                            if not (pdst == 0 and ln == P):
                                nc.scalar.memzero(scr[:, :, :, :])
                            nc.sync.dma_start(out=scr[pdst:pdst + ln, :, :, :],
                                              in_=dacc[srow:srow + ln, :, :, :])
                            for eo in (0, 1):
                                dstc = canvas[:, jb, :, eo:eo + W2D]
                                nc.vector.tensor_tensor(
                                    out=dstc, in0=dstc, in1=scr[:, eo, :, :],
                                    op=Alu.add)

                if w == 0:  # spill canvas0 to HBM (overlaps with warp 1 start)
                    for jb in range(HCB):
                        nc.sync.dma_start(out=c0_hbm[P * jb:P * jb + P, :, :],
                                          in_=canvas[:, jb, :, :])

            for _rep in range(repeat):
                do_warp(0, d_f01, t_sb[:, 0:1])
                do_warp(1, d_f10, omt[:, 0:1])

                # ---- combine ---------------------------------------------
                sl = slice(MARG_L, MARG_L + W)
                for jb in range(HCB):
                    lo = P * jb
                    o_lo = max(0, lo - ROW_OFF)
                    o_hi = min(H, lo + P - ROW_OFF)
                    if o_lo >= o_hi:
                        continue
                    cv0 = cv0_pool.tile([P, 3, W2C], fp16, tag="cv0")
                    nc.sync.dma_start(out=cv0[:, :, :], in_=c0_hbm[lo:lo + P, :, :])

                    # nhat = (1-t)*n0 + t*n1 + 1 ; m = nhat>1 ; den = nhat - m
                    tn1 = trans_pool.tile([P, W], f32, tag="tfa")
                    nc.vector.tensor_scalar(out=tn1[:, :], in0=canvas[:, jb, 2, sl],
                                            scalar1=t_sb[:, 0:1], scalar2=1.0,
                                            op0=Alu.mult, op1=Alu.add)
                    nhat = trans_pool.tile([P, W], f32, tag="tfb")
                    nc.vector.scalar_tensor_tensor(
                        out=nhat[:, :], in0=cv0[:, 2, sl], scalar=omt[:, 0:1],
                        in1=tn1[:, :], op0=Alu.mult, op1=Alu.add)
                    mgt = trans_pool.tile([P, W], f32, tag="xs")
                    nc.vector.tensor_scalar(out=mgt[:, :], in0=nhat[:, :], scalar1=1.0, scalar2=None, op0=Alu.is_gt)
                    den = trans_pool.tile([P, W], f32, tag="tfa")
                    nc.vector.tensor_tensor(out=den[:, :], in0=nhat[:, :], in1=mgt[:, :], op=Alu.subtract)
                    rec = trans_pool.tile([P, W], f32, tag="ys")
                    nc.vector.reciprocal(rec[:, :], den[:, :])

                    p_lo = o_lo + ROW_OFF - lo
                    p_hi = o_hi + ROW_OFF - lo
                    for c in range(2):
                        u = trans_pool.tile([P, W], f32, tag="tfa")
                        o0 = trans_pool.tile([P, W], f32, tag="o0")
                        o1 = trans_pool.tile([P, W], f32, tag="tfb")
                        nc.vector.tensor_scalar(out=u[:, :], in0=cv0[:, c, sl],
                                                scalar1=al0[:, 0:1], scalar2=None, op0=Alu.mult)
                        nc.vector.scalar_tensor_tensor(
                            out=o0[:, :], in0=canvas[:, jb, c, sl], scalar=al1[:, 0:1],
                            in1=u[:, :], op0=Alu.mult, op1=Alu.add)
                        nc.vector.tensor_tensor(out=o0[:, :], in0=o0[:, :], in1=rec[:, :], op=Alu.mult)
                        nc.vector.tensor_scalar(out=u[:, :], in0=cv0[:, c, sl],
                                                scalar1=be0[:, 0:1], scalar2=None, op0=Alu.mult)
                        nc.vector.scalar_tensor_tensor(
                            out=o1[:, :], in0=canvas[:, jb, c, sl], scalar=al0[:, 0:1],
                            in1=u[:, :], op0=Alu.mult, op1=Alu.add)
                        nc.vector.tensor_tensor(out=o1[:, :], in0=o1[:, :], in1=rec[:, :], op=Alu.mult)
                        nc.sync.dma_start(out=d_out0[c, o_lo:o_hi, :], in_=o0[p_lo:p_hi, :])
                        nc.sync.dma_start(out=d_out1[c, o_lo:o_hi, :], in_=o1[p_lo:p_hi, :])

    nc.finalize()
    return nc


# ---------------------------------------------------------------------------
# Entry point
# ---------------------------------------------------------------------------

def _prepare(flow_01, flow_10, t_value):
    flow_01 = np.ascontiguousarray(np.asarray(flow_01, dtype=np.float32))
    flow_10 = np.ascontiguousarray(np.asarray(flow_10, dtype=np.float32))
    t_value = np.ascontiguousarray(np.asarray(t_value, dtype=np.float32))
    n, _, H, W = flow_01.shape

    warps, ranges = _derive_plan(flow_01, flow_10, t_value)
    nc = _build_program(H, W, warps, ranges, n)

    in_maps = []
    for i in range(n):
        in_maps.append({
            "flow01": flow_01[i],
            "flow10": flow_10[i],
            "tv": np.full((P, 1), t_value[i].reshape(()), dtype=np.float32),
        })
    return nc, in_maps, n


def kernel(flow_01, flow_10, t_value):
    from concourse.bass_utils import run_bass_kernel_spmd

    nc, in_maps, n = _prepare(flow_01, flow_10, t_value)
    res = run_bass_kernel_spmd(nc, in_maps, list(range(n)))
    out0 = np.stack([res.results[i]["out0"] for i in range(n)])
    out1 = np.stack([res.results[i]["out1"] for i in range(n)])
    return out0, out1


def _make_runner(nc, in_maps, n_cores):
    """Mirror bass2jax.run_bass_via_pjrt's multi-core path, but return a
    cached jitted callable (no donation) so repeated timed runs are possible."""
    import jax
    from jax.sharding import Mesh, PartitionSpec, NamedSharding
    from jax.experimental.shard_map import shard_map
    from concourse import bass2jax, mybir

    bass2jax.install_neuronx_cc_hook()
    partition_name = nc.partition_id_tensor.name if nc.partition_id_tensor else None
    in_names, out_names, out_avals, zero_outs = [], [], [], []
    for alloc in nc.m.functions[0].allocations:
        if not isinstance(alloc, mybir.MemoryLocationSet):
            continue
        name = alloc.memorylocations[0].name
        if alloc.kind == "ExternalInput":
            if name != partition_name:
                in_names.append(name)
        elif alloc.kind == "ExternalOutput":
            shape = tuple(alloc.tensor_shape)
            dtype = mybir.dt.np(alloc.dtype)
            out_names.append(name)
            out_avals.append(jax.core.ShapedArray(shape, dtype))
            zero_outs.append(np.zeros(shape, dtype))
    n_params = len(in_names)
    all_in_names = in_names + out_names
    if partition_name is not None:
        all_in_names.append(partition_name)

    def _body(*args):
        operands = list(args)
        if partition_name is not None:
            operands.append(bass2jax.partition_id_tensor())
        return tuple(bass2jax._bass_exec_p.bind(
            *operands,
            out_avals=tuple(out_avals),
            in_names=tuple(all_in_names),
            out_names=tuple(out_names),
            lowering_input_output_aliases=(),
            sim_require_finite=True,
            sim_require_nnan=True,
            nc=nc,
        ))

    devices = jax.devices()[:n_cores]
    mesh = Mesh(np.asarray(devices), ("core",))
    in_specs = (PartitionSpec("core"),) * (n_params + len(out_names))
    out_specs = (PartitionSpec("core"),) * len(out_names)
    fn = jax.jit(shard_map(_body, mesh=mesh, in_specs=in_specs,
                           out_specs=out_specs, check_rep=False))
    per_core = [[np.asarray(m[nm]) for nm in in_names] for m in in_maps]
    concat_in = [np.concatenate([per_core[c][i] for c in range(n_cores)], axis=0)
                 for i in range(n_params)]
    concat_zero = [np.concatenate([z] * n_cores, axis=0) for z in zero_outs]
    sh = NamedSharding(mesh, PartitionSpec("core"))
    concat_in = [jax.device_put(a, sh) for a in concat_in]
    concat_zero = [jax.device_put(a, sh) for a in concat_zero]
    return fn, concat_in, concat_zero


def bench(flow_01, flow_10, t_value, iters=8):
    """Wall-clock the jitted SPMD executable; returns min per-iter ns."""
    import time
    import jax

    nc, in_maps, n = _prepare(flow_01, flow_10, t_value)
    fn, concat_in, concat_zero = _make_runner(nc, in_maps, n)
    out = fn(*concat_in, *concat_zero)
    jax.block_until_ready(out)
    times = []
    for _ in range(iters):
        t0 = time.perf_counter()
        out = fn(*concat_in, *concat_zero)
        jax.block_until_ready(out)
        times.append(time.perf_counter() - t0)
    print("bench iters (ms):", [round(t * 1e3, 2) for t in times])
    return int(min(times) * 1e9)
